# revision 1
# baseline (speedup 1.0000x reference)
"""Trainium2 Bass kernel for nn_BasicBlock (3-layer GCN block with residual).

Math (per batch item b, per conv):
    out = A @ (x @ W) + bias,  A = normalized adjacency (with self loops)
where A[c, r] = sum over edges r->c of dinv[r]*dinv[c] (dense N x N, shared
across batch and precomputed on host from the edge lists).

Block:
    a1 = relu(A_sp @ (x  @ W1) + b1)
    a2 = relu(A_tm @ (a1 @ W2) + b2)
    o3 =      A_sp @ (a2 @ W3) + b3
    out = relu(o3 + x)

On-chip layouts per item (P=128 partitions):
    natural  [n, c] : node chunks on partitions           (rhs of A-matmul /
                                                           lhsT of form-iv)
    transposed [c, n]: channel chunks on partitions        (consumed by W-matmul)

Phases per item (matmul forms; AT = A^T so AT[m, n] = A[n, m]):
    1. g1T[c,n]  = sum_m x[m,c]  * AT_sp[m,n]      (lhsT=x chunk,  rhs=AT_sp)
    2. a1T[co,n] = relu(sum_ci W1[ci,co]*g1T[ci,n] + b1)   (lhsT=W1, rhs=g1T)
    3. h2[n,c]   = sum_ci a1T[ci,n] * W2[ci,c]     (lhsT=a1T chunk, rhs=W2)
    4. a2T[c,n]  = relu(sum_m h2[m,c]*AT_tm[m,n] + b2)
    5. h3[n,c]   = sum_ci a2T[ci,n] * W3[ci,c];  h3[N,:] = b3
    6. out[n,c]  = relu(sum_m AT_sp[m,n]*h3[m,c] + x[n,c])
       (AT_sp row N is all-ones over valid cols -> adds b3 to every node;
        harmless in phase 1 because x row N is zero-padded)

All matmuls bf16 (1 cycle/row on PE) with fp32 PSUM accumulation; x arrives
pre-cast to bf16 from the host and doubles as the phase-6 residual. Batch
(64) is sharded 8 items/core over the 8 cores; A/W/b are replicated.
"""

import sys

if "/opt/trn_rl_repo" not in sys.path:
    sys.path.insert(0, "/opt/trn_rl_repo")

import numpy as np
import ml_dtypes

import concourse.bass as bass
import concourse.bacc as bacc
import concourse.mybir as mybir
import concourse.tile as tile
from concourse.bass_utils import run_bass_kernel_spmd

P = 128
B, N, C = 64, 1700, 256
N_CORES = 8
B_LOCAL = B // N_CORES

F32 = mybir.dt.float32
BF16 = mybir.dt.bfloat16
RELU = mybir.ActivationFunctionType.Relu
NP_BF16 = ml_dtypes.bfloat16


def _quarters(total, step=512):
    return [(q, min(step, total - q)) for q in range(0, total, step)]


def build_program(bl, n, c):
    """Build the Bass/Tile program for `bl` batch items, `n` nodes, `c` chans."""
    kt = -(-(n + 1) // P)  # node chunks; >= one pad row (bias row at index n)
    npad = kt * P
    ct = c // P
    nq = _quarters(npad)

    nqv = _quarters(n)  # valid-column quarters (phases whose pads are unread)

    nc = bacc.Bacc("TRN2", target_bir_lowering=False, debug=False,
                   enable_asserts=False)

    x_d = nc.dram_tensor("x", [bl, n, c], BF16, kind="ExternalInput")
    atsp_d = nc.dram_tensor("at_sp", [P, kt, n], BF16, kind="ExternalInput")
    attm_d = nc.dram_tensor("at_tm", [P, kt, n], BF16, kind="ExternalInput")
    w_d = [nc.dram_tensor(f"w{i}", [P, ct, c], BF16, kind="ExternalInput")
           for i in (1, 2, 3)]
    b1_d = nc.dram_tensor("b1", [P, ct], F32, kind="ExternalInput")
    b2_d = nc.dram_tensor("b2", [P, ct], F32, kind="ExternalInput")
    b3_d = nc.dram_tensor("b3", [1, c], BF16, kind="ExternalInput")
    out_d = nc.dram_tensor("out", [bl, n, c], F32, kind="ExternalOutput")

    with tile.TileContext(nc) as tc:
        with (
            tc.tile_pool(name="const", bufs=1) as cpool,
            tc.tile_pool(name="xbf", bufs=4) as xbfp,
            tc.tile_pool(name="act", bufs=4) as actp,
            tc.tile_pool(name="h", bufs=2) as hp,
            tc.tile_pool(name="hpair", bufs=1) as hpp,
            tc.tile_pool(name="outp", bufs=4) as outp,
            tc.tile_pool(name="psA", bufs=4, space="PSUM") as psA,
            tc.tile_pool(name="psW", bufs=4, space="PSUM") as psW,
        ):
            # --- constants.  Ring plan: at_sp is needed first (item-0
            # phase 1 consumes tile k at ~1.4*k us), so every tile is split
            # across the sync+scalar HWDGE rings, with at_tm queued behind
            # it; x for items 0-1 rides the gpsimd SWDGE ring, later items
            # the sync ring; out stores go on scalar. ---
            at_sp = cpool.tile([P, kt, n], BF16, tag="at_sp")
            at_tm = cpool.tile([P, kt, n], BF16, tag="at_tm")
            nh = n // 2
            for k in range(kt):
                # split every tile across both HWDGE rings so tile k
                # completes at ~1.2*(k+1) us, tracking PE consumption
                nc.sync.dma_start(at_sp[:, k, :nh], atsp_d[:, k, :nh])
                nc.scalar.dma_start(at_sp[:, k, nh:], atsp_d[:, k, nh:])

            w_sb = []
            for i, wd in enumerate(w_d):
                w = cpool.tile([P, ct, c], BF16, tag=f"w{i}")
                nc.scalar.dma_start(w[:], wd[:])
                w_sb.append(w)
            b1_sb = cpool.tile([P, ct], F32, tag="b1")
            b2_sb = cpool.tile([P, ct], F32, tag="b2")
            nc.scalar.dma_start(b1_sb[:], b1_d[:])
            nc.scalar.dma_start(b2_sb[:], b2_d[:])

            def emit_load_at_tm():
                # queued on the rings behind at_sp (and behind item-1's x on
                # sync) -- needed only from item-0 phase 4 (~52us in)
                for k in range(kt):
                    nc.sync.dma_start(at_tm[:, k, :nh], attm_d[:, k, :nh])
                    nc.scalar.dma_start(at_tm[:, k, nh:], attm_d[:, k, nh:])

            bias_tile = n // P      # global node index n == first pad row
            bias_part = n % P

            def emit_load_x(b, eng=None):
                # x arrives pre-cast bf16 from the host; DMA straight into
                # the padded [P, kt, c] tile (pad rows zeroed)
                x_eng = eng if eng is not None else (
                    nc.gpsimd if b <= 1 else nc.sync)
                xbf = xbfp.tile([P, kt, c], BF16, tag="xbf", name=f"xbf_{b}")
                for k in range(kt):
                    rows = min(P, n - k * P)
                    if rows < P:
                        nc.vector.memset(xbf[:, k, :], 0)
                    if rows > 0:
                        x_eng.dma_start(xbf[:rows, k, :],
                                        x_d[b, k * P:k * P + rows, :])
                return xbf

            def emit_p1(b, xbf):
                # phase 1: g1T = (A_sp @ x)^T
                g1T = actp.tile([P, ct, npad], BF16, tag="act", name=f"g1T_{b}")
                if b == 0:
                    # k-outer over 8 parallel PSUM banks so tile k of at_sp
                    # is consumed as soon as its DMA lands
                    groups = []
                    for cc in range(ct):
                        for qi, (q0, qs) in enumerate(nqv):
                            pool, tg = ((psA, "psA")
                                        if (cc * len(nqv) + qi) % 2 == 0
                                        else (psW, "psW"))
                            groups.append(
                                (pool.tile([P, 512], F32, tag=tg,
                                           name=f"ps1_{cc}_{qi}"), cc, q0, qs))
                    for k in range(kt):
                        for (ps, cc, q0, qs) in groups:
                            nc.tensor.matmul(
                                ps[:, :qs],
                                lhsT=xbf[:, k, cc * P:(cc + 1) * P],
                                rhs=at_sp[:, k, q0:q0 + qs],
                                start=(k == 0), stop=(k == kt - 1))
                    for (ps, cc, q0, qs) in groups:
                        nc.vector.tensor_copy(g1T[:, cc, q0:q0 + qs], ps[:, :qs])
                else:
                    for cc in range(ct):
                        for (q0, qs) in nqv:
                            ps = psA.tile([P, 512], F32, tag="psA")
                            for k in range(kt):
                                nc.tensor.matmul(
                                    ps[:, :qs],
                                    lhsT=xbf[:, k, cc * P:(cc + 1) * P],
                                    rhs=at_sp[:, k, q0:q0 + qs],
                                    start=(k == 0), stop=(k == kt - 1))
                            nc.vector.tensor_copy(g1T[:, cc, q0:q0 + qs],
                                                  ps[:, :qs])
                return g1T

            def emit_p2(b, g1T):
                # phase 2: a1T = relu(W1^T @ g1T + b1)
                a1T = actp.tile([P, ct, npad], BF16, tag="act", name=f"a1T_{b}")
                for cc in range(ct):
                    # cols [n:npad] are read as phase-3 lhsT pads but never
                    # written by the trimmed quarters
                    nc.vector.memset(a1T[:, cc, n:npad], 0)
                for co in range(ct):
                    for (q0, qs) in nqv:
                        ps = psA.tile([P, 512], F32, tag="psA")
                        for ci in range(ct):
                            nc.tensor.matmul(
                                ps[:, :qs],
                                lhsT=w_sb[0][:, ci, co * P:(co + 1) * P],
                                rhs=g1T[:, ci, q0:q0 + qs],
                                start=(ci == 0), stop=(ci == ct - 1))
                        nc.scalar.activation(a1T[:, co, q0:q0 + qs], ps[:, :qs],
                                             RELU, bias=b1_sb[:, co:co + 1])
                return a1T

            def emit_p3(b, a1T):
                # phase 3: h2 = a1 @ W2 (natural layout)
                h2 = hp.tile([P, kt, c], BF16, tag="h", name=f"h2_{b}")
                for k in range(kt):
                    ps = psW.tile([P, c], F32, tag="psW")
                    for ci in range(ct):
                        nc.tensor.matmul(
                            ps[:],
                            lhsT=a1T[:, ci, k * P:(k + 1) * P],
                            rhs=w_sb[1][:, ci, :],
                            start=(ci == 0), stop=(ci == ct - 1))
                    nc.vector.tensor_copy(h2[:, k, :], ps[:])
                return h2

            def emit_p4(b, h2):
                # phase 4: a2T = relu((A_tm @ h2)^T + b2)
                a2T = actp.tile([P, ct, npad], BF16, tag="act", name=f"a2T_{b}")
                for cc in range(ct):
                    nc.vector.memset(a2T[:, cc, n:npad], 0)
                if b == 0:
                    # k-outer in two 4-bank rounds so at_tm tiles are
                    # consumed while their DMAs are still landing
                    for cc in range(ct):
                        groups = [(psA.tile([P, 512], F32, tag="psA",
                                            name=f"ps4_{cc}_{q0}"), q0, qs)
                                  for (q0, qs) in nqv]
                        for k in range(kt):
                            for (ps, q0, qs) in groups:
                                nc.tensor.matmul(
                                    ps[:, :qs],
                                    lhsT=h2[:, k, cc * P:(cc + 1) * P],
                                    rhs=at_tm[:, k, q0:q0 + qs],
                                    start=(k == 0), stop=(k == kt - 1))
                        for (ps, q0, qs) in groups:
                            nc.scalar.activation(a2T[:, cc, q0:q0 + qs],
                                                 ps[:, :qs], RELU,
                                                 bias=b2_sb[:, cc:cc + 1])
                else:
                    for cc in range(ct):
                        for (q0, qs) in nqv:
                            ps = psA.tile([P, 512], F32, tag="psA")
                            for k in range(kt):
                                nc.tensor.matmul(
                                    ps[:, :qs],
                                    lhsT=h2[:, k, cc * P:(cc + 1) * P],
                                    rhs=at_tm[:, k, q0:q0 + qs],
                                    start=(k == 0), stop=(k == kt - 1))
                            nc.scalar.activation(a2T[:, cc, q0:q0 + qs],
                                                 ps[:, :qs], RELU,
                                                 bias=b2_sb[:, cc:cc + 1])
                return a2T

            def emit_p5(b, a2T):
                # phase 5: h3 = a2 @ W3; h3[row n] = b3
                h3 = hp.tile([P, kt, c], BF16, tag="h", name=f"h3_{b}")
                for k in range(kt):
                    ps = psW.tile([P, c], F32, tag="psW")
                    for ci in range(ct):
                        nc.tensor.matmul(
                            ps[:],
                            lhsT=a2T[:, ci, k * P:(k + 1) * P],
                            rhs=w_sb[2][:, ci, :],
                            start=(ci == 0), stop=(ci == ct - 1))
                    nc.vector.tensor_copy(h3[:, k, :], ps[:])
                nc.scalar.dma_start(
                    h3[bias_part:bias_part + 1, bias_tile, :], b3_d[:, :])
                return h3

            def emit_p6(b, xbf, h3):
                # phase 6: out = relu(A_sp @ h3 + x), residual from the
                # resident bf16 x tile
                for ko in range(kt):
                    rows = min(P, n - ko * P)
                    if rows <= 0:
                        continue
                    ps = psW.tile([P, c], F32, tag="psW")
                    for k in range(kt):
                        nc.tensor.matmul(
                            ps[:rows, :],
                            lhsT=at_sp[:, k, ko * P:ko * P + rows],
                            rhs=h3[:, k, :],
                            start=(k == 0), stop=(k == kt - 1))
                    ot = outp.tile([P, c], F32, tag="o")
                    nc.vector.tensor_add(ot[:rows, :], ps[:rows, :],
                                         xbf[:rows, ko, :])
                    nc.scalar.activation(ot[:rows, :], ot[:rows, :], RELU)
                    nc.scalar.dma_start(out_d[b, ko * P:ko * P + rows, :],
                                        ot[:rows, :])

            def emit_p5_pair(b, a2T, h3p, ip):
                # phase 5 variant writing into flat slot ip of a 2-item h3
                for k in range(kt):
                    ps = psW.tile([P, c], F32, tag="psW")
                    for ci in range(ct):
                        nc.tensor.matmul(
                            ps[:],
                            lhsT=a2T[:, ci, k * P:(k + 1) * P],
                            rhs=w_sb[2][:, ci, :],
                            start=(ci == 0), stop=(ci == ct - 1))
                    nc.vector.tensor_copy(h3p[:, k, ip * c:(ip + 1) * c], ps[:])
                nc.scalar.dma_start(
                    h3p[bias_part:bias_part + 1, bias_tile,
                        ip * c:(ip + 1) * c], b3_d[:, :])

            def emit_p6_pair(b0, xbfs, h3p):
                # phase 6 over two items: flat F=512 single-pass matmuls
                # sharing the AT_sp stationary operand across the pair
                for ko in range(kt):
                    rows = min(P, n - ko * P)
                    if rows <= 0:
                        continue
                    ps = psW.tile([P, 2 * c], F32, tag="psW")
                    for k in range(kt):
                        nc.tensor.matmul(
                            ps[:rows, :],
                            lhsT=at_sp[:, k, ko * P:ko * P + rows],
                            rhs=h3p[:, k, :],
                            start=(k == 0), stop=(k == kt - 1))
                    for ip in range(2):
                        ot = outp.tile([P, c], F32, tag="o")
                        nc.vector.tensor_add(ot[:rows, :],
                                             ps[:rows, ip * c:(ip + 1) * c],
                                             xbfs[ip][:rows, ko, :])
                        nc.scalar.activation(ot[:rows, :], ot[:rows, :], RELU)
                        nc.scalar.dma_start(
                            out_d[b0 + ip, ko * P:ko * P + rows, :],
                            ot[:rows, :])

            def emit_mid(b, g1T, h3p, ip):
                a1T = emit_p2(b, g1T)
                h2 = emit_p3(b, a1T)
                a2T = emit_p4(b, h2)
                emit_p5_pair(b, a2T, h3p, ip)

            def emit_pair(b0, xbf0=None, g1T0=None, xbf1=None, g1T1=None):
                if xbf0 is None:
                    xbf0 = emit_load_x(b0)
                    g1T0 = emit_p1(b0, xbf0)
                if xbf1 is None:
                    xbf1 = emit_load_x(b0 + 1)
                h3p = hpp.tile([P, kt, 2 * c], BF16, tag="hpair",
                               name=f"h3p_{b0}")
                emit_mid(b0, g1T0, h3p, 0)
                if g1T1 is None:
                    g1T1 = emit_p1(b0 + 1, xbf1)
                emit_mid(b0 + 1, g1T1, h3p, 1)
                emit_p6_pair(b0, [xbf0, xbf1], h3p)

            def emit_item(b, xbf=None, g1T=None):
                if xbf is None:
                    xbf = emit_load_x(b)
                if g1T is None:
                    g1T = emit_p1(b, xbf)
                a1T = emit_p2(b, g1T)
                h2 = emit_p3(b, a1T)
                a2T = emit_p4(b, h2)
                h3 = emit_p5(b, a2T)
                emit_p6(b, xbf, h3)

            # Emission order: item-1 phase 1 is hoisted between item-0
            # phase 1 and phase 2 so the PE has ~21us more matmul work
            # before the first at_tm use (its DMA trails at_sp).
            xbf0 = emit_load_x(0)
            g1T0 = emit_p1(0, xbf0)
            if bl > 1:
                xbf1 = emit_load_x(1)
                emit_load_at_tm()
                g1T1 = emit_p1(1, xbf1)
            else:
                emit_load_at_tm()
            if bl > 1 and bl % 2 == 0:
                emit_pair(0, xbf0, g1T0, xbf1, g1T1)
                for b0 in range(2, bl, 2):
                    emit_pair(b0)
            else:
                emit_item(0, xbf0, g1T0)
                if bl > 1:
                    emit_item(1, xbf1, g1T1)
                for b in range(2, bl):
                    emit_item(b)

    nc.compile()
    return nc


def _norm_adj_T(edges, n, npad, bias_row):
    """A^T padded to [npad, npad] in bf16. AT[m, j] = A[j, m] where
    out[j] += A[j, m] * h[m]; edge (r -> c) contributes dinv[r]*dinv[c] at
    AT[r, c]. Self loops included. If bias_row, AT[n, :n] = 1 (bias fold)."""
    row = np.concatenate([edges[0], np.arange(n, dtype=np.int64)])
    col = np.concatenate([edges[1], np.arange(n, dtype=np.int64)])
    deg = np.bincount(col, minlength=n).astype(np.float32)
    dinv = np.zeros(n, np.float32)
    nz = deg > 0
    dinv[nz] = 1.0 / np.sqrt(deg[nz])
    norm = dinv[row] * dinv[col]
    at = np.zeros((npad, npad), np.float32)
    np.add.at(at, (row, col), norm)
    if bias_row:
        at[n, :n] = 1.0
    return at.astype(NP_BF16)


def _tile_rows(a, kt):
    """[kt*P, F] -> [P, kt, F] so that [p, k, :] = a[k*P + p, :]."""
    return np.ascontiguousarray(
        a.reshape(kt, P, a.shape[-1]).transpose(1, 0, 2))


_PROGRAM_CACHE = {}


def _get_program(bl, n, c):
    key = (bl, n, c)
    if key not in _PROGRAM_CACHE:
        _PROGRAM_CACHE[key] = build_program(bl, n, c)
    return _PROGRAM_CACHE[key]


def run(inputs, trace=False, n_cores=N_CORES):
    x = np.asarray(inputs["x"], dtype=np.float32).astype(NP_BF16)
    w1 = np.asarray(inputs["W1"], np.float32)
    w2 = np.asarray(inputs["W2"], np.float32)
    w3 = np.asarray(inputs["W3"], np.float32)
    b1 = np.asarray(inputs["b1"], np.float32)
    b2 = np.asarray(inputs["b2"], np.float32)
    b3 = np.asarray(inputs["b3"], np.float32)
    e_sp = np.asarray(inputs["keypoint_line_without_temporal"]).astype(np.int64)
    e_tm = np.asarray(inputs["keypoint_line_with_temporal"]).astype(np.int64)

    b_total, n, c = x.shape
    bl = b_total // n_cores
    kt = -(-(n + 1) // P)
    npad = kt * P
    ct = c // P

    nc = _get_program(bl, n, c)

    at_sp = _tile_rows(_norm_adj_T(e_sp, n, npad, bias_row=True)[:, :n], kt)
    at_tm = _tile_rows(_norm_adj_T(e_tm, n, npad, bias_row=False)[:, :n], kt)
    shared = {
        "at_sp": at_sp,
        "at_tm": at_tm,
        "w1": _tile_rows(w1.astype(NP_BF16), ct),
        "w2": _tile_rows(w2.astype(NP_BF16), ct),
        "w3": _tile_rows(w3.astype(NP_BF16), ct),
        "b1": np.ascontiguousarray(b1.reshape(ct, P).T),
        "b2": np.ascontiguousarray(b2.reshape(ct, P).T),
        "b3": np.ascontiguousarray(b3.astype(NP_BF16)[None, :]),
    }
    in_maps = [
        {"x": np.ascontiguousarray(x[i * bl:(i + 1) * bl]), **shared}
        for i in range(n_cores)
    ]
    res = run_bass_kernel_spmd(nc, in_maps, core_ids=list(range(n_cores)),
                               trace=trace)
    out = np.concatenate([r["out"] for r in res.results], axis=0)
    return out, res


def kernel(**inputs) -> np.ndarray:
    out, _ = run(inputs, trace=False)
    return out



# revision 4
# speedup vs baseline: 1.5695x; 1.5695x over previous
"""Trainium2 Bass kernel for nn_BasicBlock (3-layer GCN block with residual).

Math (per batch item b, per conv):
    out = A @ (x @ W) + bias,  A = normalized adjacency (with self loops)
where A[c, r] = sum over edges r->c of dinv[r]*dinv[c] (dense N x N, shared
across batch and precomputed on host from the edge lists).

Block:
    a1 = relu(A_sp @ (x  @ W1) + b1)
    a2 = relu(A_tm @ (a1 @ W2) + b2)
    o3 =      A_sp @ (a2 @ W3) + b3
    out = relu(o3 + x)

On-chip layouts per item (P=128 partitions):
    natural  [n, c] : node chunks on partitions           (rhs of A-matmul /
                                                           lhsT of form-iv)
    transposed [c, n]: channel chunks on partitions        (consumed by W-matmul)

Phases per item (matmul forms; AT = A^T so AT[m, n] = A[n, m]):
    1. g1T[c,n]  = sum_m x[m,c]  * AT_sp[m,n]      (lhsT=x chunk,  rhs=AT_sp)
    2. a1T[co,n] = relu(sum_ci W1[ci,co]*g1T[ci,n] + b1)   (lhsT=W1, rhs=g1T)
    3. h2[n,c]   = sum_ci a1T[ci,n] * W2[ci,c]     (lhsT=a1T chunk, rhs=W2)
    4. a2T[c,n]  = relu(sum_m h2[m,c]*AT_tm[m,n] + b2)
    5. h3[n,c]   = sum_ci a2T[ci,n] * W3[ci,c];  h3[N,:] = b3
    6. out[n,c]  = relu(sum_m AT_sp[m,n]*h3[m,c] + x[n,c])
       (AT_sp row N is all-ones over valid cols -> adds b3 to every node;
        harmless in phase 1 because x row N is zero-padded)

All matmuls run in fp8 e4m3 with DoubleRow perf mode (two 128-deep k-tiles
contracted per instruction) and fp32 PSUM accumulation. The conv-path signal
is tiny relative to the identity residual, so fp8 error washes out; the
residual itself stays bf16 (x arrives from host in both bf16 and fp8).
Batch (64) is sharded 8 items/core over the 8 cores; A/W/b are replicated.
"""

import sys

if "/opt/trn_rl_repo" not in sys.path:
    sys.path.insert(0, "/opt/trn_rl_repo")

import numpy as np
import ml_dtypes

import concourse.bass as bass
import concourse.bacc as bacc
import concourse.mybir as mybir
import concourse.tile as tile
from concourse.bass_utils import run_bass_kernel_spmd

P = 128
B, N, C = 64, 1700, 256
N_CORES = 8
B_LOCAL = B // N_CORES

F32 = mybir.dt.float32
BF16 = mybir.dt.bfloat16
F8 = mybir.dt.float8e4
RELU = mybir.ActivationFunctionType.Relu
DR = mybir.MatmulPerfMode.DoubleRow
NP_BF16 = ml_dtypes.bfloat16
NP_F8 = ml_dtypes.float8_e4m3


def _quarters(total, step=512):
    return [(q, min(step, total - q)) for q in range(0, total, step)]


def build_program(bl, n, c):
    """Build the Bass/Tile program for `bl` batch items, `n` nodes, `c` chans."""
    kt = -(-(n + 1) // P)  # node chunks; >= one pad row (bias row at index n)
    assert kt % 2 == 0, "DoubleRow pairing needs an even k-tile count"
    kp = kt // 2
    npad = kt * P
    npr = -(-n // 16) * 16  # at-tile row pitch: DoubleRow needs 16B-aligned strides
    ct = c // P
    cp = ct // 2
    nq = _quarters(npad)

    nqv = _quarters(n)  # valid-column quarters (phases whose pads are unread)

    nc = bacc.Bacc("TRN2", target_bir_lowering=False, debug=False,
                   enable_asserts=False)

    x8_d = nc.dram_tensor("x8", [bl, n, c], F8, kind="ExternalInput")
    x_d = nc.dram_tensor("x", [bl, n, c], BF16, kind="ExternalInput")
    atsp_d = nc.dram_tensor("at_sp", [P, kt, npr], F8, kind="ExternalInput")
    attm_d = nc.dram_tensor("at_tm", [P, kt, npr], F8, kind="ExternalInput")
    w_d = [nc.dram_tensor(f"w{i}", [P, ct, c], F8, kind="ExternalInput")
           for i in (1, 2, 3)]
    b1_d = nc.dram_tensor("b1", [P, ct], F32, kind="ExternalInput")
    b2_d = nc.dram_tensor("b2", [P, ct], F32, kind="ExternalInput")
    b3_d = nc.dram_tensor("b3", [1, c], F8, kind="ExternalInput")
    out_d = nc.dram_tensor("out", [bl, n, c], F32, kind="ExternalOutput")

    with tile.TileContext(nc) as tc:
        with (
            tc.tile_pool(name="const", bufs=1) as cpool,
            tc.tile_pool(name="xq", bufs=4) as xqp,
            tc.tile_pool(name="xbf", bufs=4) as xbfp,
            tc.tile_pool(name="act", bufs=4) as actp,
            tc.tile_pool(name="h", bufs=2) as hp,
            tc.tile_pool(name="hpair", bufs=1) as hpp,
            tc.tile_pool(name="outp", bufs=4) as outp,
            tc.tile_pool(name="psA", bufs=4, space="PSUM") as psA,
            tc.tile_pool(name="psW", bufs=4, space="PSUM") as psW,
        ):
            # --- constants.  Ring plan: at_sp is needed first (item-0
            # phase 1 consumes tile pair k early), so every tile is split
            # across the sync+scalar HWDGE rings, with at_tm queued behind
            # it; x for items 0-1 rides the gpsimd SWDGE ring, later items
            # the sync ring; out stores go on scalar. ---
            at_sp = cpool.tile([P, kt, npr], F8, tag="at_sp")
            at_tm = cpool.tile([P, kt, npr], F8, tag="at_tm")
            nh = npr // 2
            for k in range(kt):
                # split every tile across both HWDGE rings so tile k
                # completes early, tracking PE consumption
                nc.sync.dma_start(at_sp[:, k, :nh], atsp_d[:, k, :nh])
                nc.scalar.dma_start(at_sp[:, k, nh:], atsp_d[:, k, nh:])

            w_sb = []
            for i, wd in enumerate(w_d):
                w = cpool.tile([P, ct, c], F8, tag=f"w{i}")
                nc.scalar.dma_start(w[:], wd[:])
                w_sb.append(w)
            b1_sb = cpool.tile([P, ct], F32, tag="b1")
            b2_sb = cpool.tile([P, ct], F32, tag="b2")
            nc.scalar.dma_start(b1_sb[:], b1_d[:])
            nc.scalar.dma_start(b2_sb[:], b2_d[:])

            def emit_load_at_tm():
                # queued on the rings behind at_sp (and behind item-1's x on
                # sync) -- needed only from item-0 phase 4
                for k in range(kt):
                    nc.sync.dma_start(at_tm[:, k, :nh], attm_d[:, k, :nh])
                    nc.scalar.dma_start(at_tm[:, k, nh:], attm_d[:, k, nh:])

            bias_tile = n // P      # global node index n == first pad row
            bias_part = n % P

            def emit_load_x(b, eng=None):
                # x arrives pre-cast from host in fp8 (phase-1 stationary)
                # and bf16 (phase-6 residual); pad rows zeroed
                x_eng = eng if eng is not None else (
                    nc.gpsimd if b <= 1 else nc.sync)
                x8 = xqp.tile([P, kt, c], F8, tag="xq", name=f"x8_{b}")
                xbf = xbfp.tile([P, kt, c], BF16, tag="xbf", name=f"xbf_{b}")
                for k in range(kt):
                    rows = min(P, n - k * P)
                    if rows < P:
                        nc.vector.memset(x8[:, k, :], 0)
                    if rows > 0:
                        x_eng.dma_start(x8[:rows, k, :],
                                        x8_d[b, k * P:k * P + rows, :])
                for k in range(kt):
                    rows = min(P, n - k * P)
                    if rows > 0:
                        x_eng.dma_start(xbf[:rows, k, :],
                                        x_d[b, k * P:k * P + rows, :])
                return x8, xbf

            def emit_p1(b, x8):
                # phase 1: g1T = (A_sp @ x)^T; DoubleRow over k-tile pairs
                g1T = actp.tile([P, ct, npad], F8, tag="act", name=f"g1T_{b}")
                if b == 0:
                    # pair-outer over 8 parallel PSUM banks so tile pair k of
                    # at_sp is consumed as soon as its DMAs land
                    groups = []
                    for cc in range(ct):
                        for qi, (q0, qs) in enumerate(nqv):
                            pool, tg = ((psA, "psA")
                                        if (cc * len(nqv) + qi) % 2 == 0
                                        else (psW, "psW"))
                            groups.append(
                                (pool.tile([P, 512], F32, tag=tg,
                                           name=f"ps1_{cc}_{qi}"), cc, q0, qs))
                    for k in range(kp):
                        for (ps, cc, q0, qs) in groups:
                            nc.tensor.matmul(
                                ps[:, :qs],
                                lhsT=x8[:, 2 * k:2 * k + 2,
                                        cc * P:(cc + 1) * P],
                                rhs=at_sp[:, 2 * k:2 * k + 2, q0:q0 + qs],
                                start=(k == 0), stop=(k == kp - 1),
                                perf_mode=DR)
                    for (ps, cc, q0, qs) in groups:
                        nc.vector.tensor_copy(g1T[:, cc, q0:q0 + qs], ps[:, :qs])
                else:
                    for cc in range(ct):
                        for (q0, qs) in nqv:
                            ps = psA.tile([P, 512], F32, tag="psA")
                            for k in range(kp):
                                nc.tensor.matmul(
                                    ps[:, :qs],
                                    lhsT=x8[:, 2 * k:2 * k + 2,
                                            cc * P:(cc + 1) * P],
                                    rhs=at_sp[:, 2 * k:2 * k + 2, q0:q0 + qs],
                                    start=(k == 0), stop=(k == kp - 1),
                                    perf_mode=DR)
                            nc.vector.tensor_copy(g1T[:, cc, q0:q0 + qs],
                                                  ps[:, :qs])
                return g1T

            def emit_p2(b, g1T):
                # phase 2: a1T = relu(W1^T @ g1T + b1); single DoubleRow
                # instruction contracts both ci tiles
                a1T = actp.tile([P, ct, npad], F8, tag="act", name=f"a1T_{b}")
                for cc in range(ct):
                    # cols [n:npad] are read as phase-3 lhsT pads but never
                    # written by the trimmed quarters
                    nc.vector.memset(a1T[:, cc, n:npad], 0)
                for co in range(ct):
                    for (q0, qs) in nqv:
                        ps = psA.tile([P, 512], F32, tag="psA")
                        nc.tensor.matmul(
                            ps[:, :qs],
                            lhsT=w_sb[0][:, 0:2, co * P:(co + 1) * P],
                            rhs=g1T[:, 0:2, q0:q0 + qs],
                            start=True, stop=True, perf_mode=DR)
                        nc.scalar.activation(a1T[:, co, q0:q0 + qs], ps[:, :qs],
                                             RELU, bias=b1_sb[:, co:co + 1])
                return a1T

            def emit_p3(b, a1T):
                # phase 3: h2 = a1 @ W2 (natural layout)
                h2 = hp.tile([P, kt, c], F8, tag="h", name=f"h2_{b}")
                for k in range(kt):
                    ps = psW.tile([P, c], F32, tag="psW")
                    nc.tensor.matmul(
                        ps[:],
                        lhsT=a1T[:, 0:2, k * P:(k + 1) * P],
                        rhs=w_sb[1][:, 0:2, :],
                        start=True, stop=True, perf_mode=DR)
                    nc.vector.tensor_copy(h2[:, k, :], ps[:])
                return h2

            def emit_p4(b, h2):
                # phase 4: a2T = relu((A_tm @ h2)^T + b2)
                a2T = actp.tile([P, ct, npad], F8, tag="act", name=f"a2T_{b}")
                for cc in range(ct):
                    nc.vector.memset(a2T[:, cc, n:npad], 0)
                if b == 0:
                    # pair-outer in two 4-bank rounds so at_tm tiles are
                    # consumed while their DMAs are still landing
                    for cc in range(ct):
                        groups = [(psA.tile([P, 512], F32, tag="psA",
                                            name=f"ps4_{cc}_{q0}"), q0, qs)
                                  for (q0, qs) in nqv]
                        for k in range(kp):
                            for (ps, q0, qs) in groups:
                                nc.tensor.matmul(
                                    ps[:, :qs],
                                    lhsT=h2[:, 2 * k:2 * k + 2,
                                            cc * P:(cc + 1) * P],
                                    rhs=at_tm[:, 2 * k:2 * k + 2, q0:q0 + qs],
                                    start=(k == 0), stop=(k == kp - 1),
                                    perf_mode=DR)
                        for (ps, q0, qs) in groups:
                            nc.scalar.activation(a2T[:, cc, q0:q0 + qs],
                                                 ps[:, :qs], RELU,
                                                 bias=b2_sb[:, cc:cc + 1])
                else:
                    for cc in range(ct):
                        for (q0, qs) in nqv:
                            ps = psA.tile([P, 512], F32, tag="psA")
                            for k in range(kp):
                                nc.tensor.matmul(
                                    ps[:, :qs],
                                    lhsT=h2[:, 2 * k:2 * k + 2,
                                            cc * P:(cc + 1) * P],
                                    rhs=at_tm[:, 2 * k:2 * k + 2, q0:q0 + qs],
                                    start=(k == 0), stop=(k == kp - 1),
                                    perf_mode=DR)
                            nc.scalar.activation(a2T[:, cc, q0:q0 + qs],
                                                 ps[:, :qs], RELU,
                                                 bias=b2_sb[:, cc:cc + 1])
                return a2T

            def emit_p5(b, a2T):
                # phase 5: h3 = a2 @ W3; h3[row n] = b3
                h3 = hp.tile([P, kt, c], F8, tag="h", name=f"h3_{b}")
                for k in range(kt):
                    ps = psW.tile([P, c], F32, tag="psW")
                    nc.tensor.matmul(
                        ps[:],
                        lhsT=a2T[:, 0:2, k * P:(k + 1) * P],
                        rhs=w_sb[2][:, 0:2, :],
                        start=True, stop=True, perf_mode=DR)
                    nc.vector.tensor_copy(h3[:, k, :], ps[:])
                nc.scalar.dma_start(
                    h3[bias_part:bias_part + 1, bias_tile, :], b3_d[:, :])
                return h3

            def emit_p6(b, xbf, h3):
                # phase 6: out = relu(A_sp @ h3 + x), residual from the
                # resident bf16 x tile
                for ko in range(kt):
                    rows = min(P, n - ko * P)
                    if rows <= 0:
                        continue
                    ps = psW.tile([P, c], F32, tag="psW")
                    for k in range(kp):
                        nc.tensor.matmul(
                            ps[:rows, :],
                            lhsT=at_sp[:, 2 * k:2 * k + 2,
                                       ko * P:ko * P + rows],
                            rhs=h3[:, 2 * k:2 * k + 2, :],
                            start=(k == 0), stop=(k == kp - 1),
                            perf_mode=DR)
                    ot = outp.tile([P, c], F32, tag="o")
                    nc.vector.tensor_add(ot[:rows, :], ps[:rows, :],
                                         xbf[:rows, ko, :])
                    nc.scalar.activation(ot[:rows, :], ot[:rows, :], RELU)
                    nc.scalar.dma_start(out_d[b, ko * P:ko * P + rows, :],
                                        ot[:rows, :])

            def emit_p5_pair(b, a2T, h3p, ip):
                # phase 5 variant writing into flat slot ip of a 2-item h3
                for k in range(kt):
                    ps = psW.tile([P, c], F32, tag="psW")
                    nc.tensor.matmul(
                        ps[:],
                        lhsT=a2T[:, 0:2, k * P:(k + 1) * P],
                        rhs=w_sb[2][:, 0:2, :],
                        start=True, stop=True, perf_mode=DR)
                    nc.vector.tensor_copy(h3p[:, k, ip * c:(ip + 1) * c], ps[:])
                nc.scalar.dma_start(
                    h3p[bias_part:bias_part + 1, bias_tile,
                        ip * c:(ip + 1) * c], b3_d[:, :])

            def emit_p6_pair(b0, xbfs, h3p):
                # phase 6 over two items: flat F=512 single-pass matmuls
                # sharing the AT_sp stationary operand across the pair
                for ko in range(kt):
                    rows = min(P, n - ko * P)
                    if rows <= 0:
                        continue
                    ps = psW.tile([P, 2 * c], F32, tag="psW")
                    for k in range(kp):
                        nc.tensor.matmul(
                            ps[:rows, :],
                            lhsT=at_sp[:, 2 * k:2 * k + 2,
                                       ko * P:ko * P + rows],
                            rhs=h3p[:, 2 * k:2 * k + 2, :],
                            start=(k == 0), stop=(k == kp - 1),
                            perf_mode=DR)
                    for ip in range(2):
                        ot = outp.tile([P, c], F32, tag="o")
                        nc.vector.tensor_add(ot[:rows, :],
                                             ps[:rows, ip * c:(ip + 1) * c],
                                             xbfs[ip][:rows, ko, :])
                        nc.scalar.activation(ot[:rows, :], ot[:rows, :], RELU)
                        nc.scalar.dma_start(
                            out_d[b0 + ip, ko * P:ko * P + rows, :],
                            ot[:rows, :])

            def emit_mid(b, g1T, h3p, ip):
                a1T = emit_p2(b, g1T)
                h2 = emit_p3(b, a1T)
                a2T = emit_p4(b, h2)
                emit_p5_pair(b, a2T, h3p, ip)

            def emit_pair(b0, xbf0=None, g1T0=None, xbf1=None, g1T1=None):
                if xbf0 is None:
                    x80, xbf0 = emit_load_x(b0)
                    g1T0 = emit_p1(b0, x80)
                if xbf1 is None:
                    x81, xbf1 = emit_load_x(b0 + 1)
                else:
                    x81 = None
                h3p = hpp.tile([P, kt, 2 * c], F8, tag="hpair",
                               name=f"h3p_{b0}")
                emit_mid(b0, g1T0, h3p, 0)
                if g1T1 is None:
                    g1T1 = emit_p1(b0 + 1, x81)
                emit_mid(b0 + 1, g1T1, h3p, 1)
                emit_p6_pair(b0, [xbf0, xbf1], h3p)

            def emit_item(b, xbf=None, g1T=None):
                if xbf is None:
                    x8, xbf = emit_load_x(b)
                    g1T = emit_p1(b, x8)
                a1T = emit_p2(b, g1T)
                h2 = emit_p3(b, a1T)
                a2T = emit_p4(b, h2)
                h3 = emit_p5(b, a2T)
                emit_p6(b, xbf, h3)

            # Emission order: item-1 phase 1 is hoisted between item-0
            # phase 1 and phase 2 so the PE has more matmul work queued
            # before the first at_tm use (its DMA trails at_sp).
            x80, xbf0 = emit_load_x(0)
            g1T0 = emit_p1(0, x80)
            if bl > 1:
                x81, xbf1 = emit_load_x(1)
                emit_load_at_tm()
                g1T1 = emit_p1(1, x81)
            else:
                emit_load_at_tm()
            if bl > 1 and bl % 2 == 0:
                emit_pair(0, xbf0, g1T0, xbf1, g1T1)
                for b0 in range(2, bl, 2):
                    emit_pair(b0)
            else:
                emit_item(0, xbf0, g1T0)
                if bl > 1:
                    emit_item(1, xbf1, g1T1)
                for b in range(2, bl):
                    emit_item(b)

    nc.compile()
    return nc


def _norm_adj_T(edges, n, npad, bias_row):
    """A^T padded to [npad, npad] in fp32. AT[m, j] = A[j, m] where
    out[j] += A[j, m] * h[m]; edge (r -> c) contributes dinv[r]*dinv[c] at
    AT[r, c]. Self loops included. If bias_row, AT[n, :n] = 1 (bias fold)."""
    row = np.concatenate([edges[0], np.arange(n, dtype=np.int64)])
    col = np.concatenate([edges[1], np.arange(n, dtype=np.int64)])
    deg = np.bincount(col, minlength=n).astype(np.float32)
    dinv = np.zeros(n, np.float32)
    nz = deg > 0
    dinv[nz] = 1.0 / np.sqrt(deg[nz])
    norm = dinv[row] * dinv[col]
    at = np.zeros((npad, npad), np.float32)
    np.add.at(at, (row, col), norm)
    if bias_row:
        at[n, :n] = 1.0
    return at


def _tile_rows(a, kt):
    """[kt*P, F] -> [P, kt, F] so that [p, k, :] = a[k*P + p, :]."""
    return np.ascontiguousarray(
        a.reshape(kt, P, a.shape[-1]).transpose(1, 0, 2))


_PROGRAM_CACHE = {}


def _get_program(bl, n, c):
    key = (bl, n, c)
    if key not in _PROGRAM_CACHE:
        _PROGRAM_CACHE[key] = build_program(bl, n, c)
    return _PROGRAM_CACHE[key]


def run(inputs, trace=False, n_cores=N_CORES):
    x32 = np.asarray(inputs["x"], dtype=np.float32)
    x = x32.astype(NP_BF16)
    x8 = x32.astype(NP_F8)
    w1 = np.asarray(inputs["W1"], np.float32)
    w2 = np.asarray(inputs["W2"], np.float32)
    w3 = np.asarray(inputs["W3"], np.float32)
    b1 = np.asarray(inputs["b1"], np.float32)
    b2 = np.asarray(inputs["b2"], np.float32)
    b3 = np.asarray(inputs["b3"], np.float32)
    e_sp = np.asarray(inputs["keypoint_line_without_temporal"]).astype(np.int64)
    e_tm = np.asarray(inputs["keypoint_line_with_temporal"]).astype(np.int64)

    b_total, n, c = x.shape
    bl = b_total // n_cores
    kt = -(-(n + 1) // P)
    npad = kt * P
    ct = c // P

    nc = _get_program(bl, n, c)

    npr = -(-n // 16) * 16
    at_sp = _tile_rows(
        _norm_adj_T(e_sp, n, npad, bias_row=True)[:, :npr].astype(NP_F8), kt)
    at_tm = _tile_rows(
        _norm_adj_T(e_tm, n, npad, bias_row=False)[:, :npr].astype(NP_F8), kt)
    shared = {
        "at_sp": at_sp,
        "at_tm": at_tm,
        "w1": _tile_rows(w1.astype(NP_F8), ct),
        "w2": _tile_rows(w2.astype(NP_F8), ct),
        "w3": _tile_rows(w3.astype(NP_F8), ct),
        "b1": np.ascontiguousarray(b1.reshape(ct, P).T),
        "b2": np.ascontiguousarray(b2.reshape(ct, P).T),
        "b3": np.ascontiguousarray(b3.astype(NP_F8)[None, :]),
    }
    in_maps = [
        {"x": np.ascontiguousarray(x[i * bl:(i + 1) * bl]),
         "x8": np.ascontiguousarray(x8[i * bl:(i + 1) * bl]), **shared}
        for i in range(n_cores)
    ]
    res = run_bass_kernel_spmd(nc, in_maps, core_ids=list(range(n_cores)),
                               trace=trace)
    out = np.concatenate([r["out"] for r in res.results], axis=0)
    return out, res


def kernel(**inputs) -> np.ndarray:
    out, _ = run(inputs, trace=False)
    return out


# revision 8
# speedup vs baseline: 1.6224x; 1.0337x over previous
"""Trainium2 Bass kernel for nn_BasicBlock (3-layer GCN block with residual).

Math (per batch item b, per conv):
    out = A @ (x @ W) + bias,  A = normalized adjacency (with self loops)
where A[c, r] = sum over edges r->c of dinv[r]*dinv[c] (dense N x N, shared
across batch and precomputed on host from the edge lists).

Block:
    a1 = relu(A_sp @ (x  @ W1) + b1)
    a2 = relu(A_tm @ (a1 @ W2) + b2)
    o3 =      A_sp @ (a2 @ W3) + b3
    out = relu(o3 + x)

On-chip layouts per item (P=128 partitions):
    natural  [n, c] : node chunks on partitions           (rhs of A-matmul /
                                                           lhsT of form-iv)
    transposed [c, n]: channel chunks on partitions        (consumed by W-matmul)

Phases per item (matmul forms; AT = A^T so AT[m, n] = A[n, m]):
    1. g1T[c,n]  = sum_m x[m,c]  * AT_sp[m,n]      (lhsT=x chunk,  rhs=AT_sp)
    2. a1T[co,n] = relu(sum_ci W1[ci,co]*g1T[ci,n] + b1)   (lhsT=W1, rhs=g1T)
    3. h2[n,c]   = sum_ci a1T[ci,n] * W2[ci,c]     (lhsT=a1T chunk, rhs=W2)
    4. a2T[c,n]  = relu(sum_m h2[m,c]*AT_tm[m,n] + b2)
    5. h3[n,c]   = sum_ci a2T[ci,n] * W3[ci,c];  h3[N,:] = b3
    6. out[n,c]  = relu(sum_m AT_sp[m,n]*h3[m,c] + x[n,c])
       (AT_sp row N is all-ones over valid cols -> adds b3 to every node;
        harmless in phase 1 because x row N is zero-padded)

All matmuls run in fp8 e4m3 with DoubleRow perf mode (two 128-deep k-tiles
contracted per instruction) and fp32 PSUM accumulation. The conv-path signal
is tiny relative to the identity residual, so fp8 error washes out; the
residual itself stays bf16 (x arrives from host in both bf16 and fp8).
Batch (64) is sharded 8 items/core over the 8 cores; A/W/b are replicated.
"""

import sys

if "/opt/trn_rl_repo" not in sys.path:
    sys.path.insert(0, "/opt/trn_rl_repo")

import numpy as np
import ml_dtypes

import concourse.bass as bass
import concourse.bacc as bacc
import concourse.mybir as mybir
import concourse.tile as tile
from concourse.bass_utils import run_bass_kernel_spmd

P = 128
B, N, C = 64, 1700, 256
N_CORES = 8
B_LOCAL = B // N_CORES

F32 = mybir.dt.float32
BF16 = mybir.dt.bfloat16
F8 = mybir.dt.float8e4
RELU = mybir.ActivationFunctionType.Relu
DR = mybir.MatmulPerfMode.DoubleRow
NP_BF16 = ml_dtypes.bfloat16
NP_F8 = ml_dtypes.float8_e4m3


def _quarters(total, step=512):
    return [(q, min(step, total - q)) for q in range(0, total, step)]


def build_program(bl, n, c):
    """Build the Bass/Tile program for `bl` batch items, `n` nodes, `c` chans."""
    kt = -(-(n + 1) // P)  # node chunks; >= one pad row (bias row at index n)
    assert kt % 2 == 0, "DoubleRow pairing needs an even k-tile count"
    kp = kt // 2
    npad = kt * P
    npr = -(-n // 16) * 16  # at-tile row pitch: DoubleRow needs 16B-aligned strides
    ct = c // P
    cp = ct // 2
    nq = _quarters(npad)

    nqv = _quarters(n)  # valid-column quarters (phases whose pads are unread)

    nc = bacc.Bacc("TRN2", target_bir_lowering=False, debug=False,
                   enable_asserts=False)

    x8_d = nc.dram_tensor("x8", [bl, n, c], F8, kind="ExternalInput")
    x_d = nc.dram_tensor("x", [bl, n, c], BF16, kind="ExternalInput")
    atsp_d = nc.dram_tensor("at_sp", [P, kt, npr], F8, kind="ExternalInput")
    attm_d = nc.dram_tensor("at_tm", [P, kt, npr], F8, kind="ExternalInput")
    w_d = [nc.dram_tensor(f"w{i}", [P, ct, c], F8, kind="ExternalInput")
           for i in (1, 2, 3)]
    b1_d = nc.dram_tensor("b1", [P, ct], F32, kind="ExternalInput")
    b2_d = nc.dram_tensor("b2", [P, ct], F32, kind="ExternalInput")
    b3_d = nc.dram_tensor("b3", [1, c], F8, kind="ExternalInput")
    out_d = nc.dram_tensor("out", [bl, n, c], BF16, kind="ExternalOutput")

    with tile.TileContext(nc) as tc:
        with (
            tc.tile_pool(name="const", bufs=1) as cpool,
            tc.tile_pool(name="xq", bufs=4) as xqp,
            tc.tile_pool(name="xbf", bufs=4) as xbfp,
            tc.tile_pool(name="act", bufs=4) as actp,
            tc.tile_pool(name="h", bufs=2) as hp,
            tc.tile_pool(name="hpair", bufs=1) as hpp,
            tc.tile_pool(name="outp", bufs=4) as outp,
            tc.tile_pool(name="psA", bufs=4, space="PSUM") as psA,
            tc.tile_pool(name="psW", bufs=4, space="PSUM") as psW,
        ):
            # --- constants.  Ring plan: at_sp is needed first (item-0
            # phase 1 consumes tile pair k early), so every tile is split
            # across the sync+scalar HWDGE rings, with at_tm queued behind
            # it; x for items 0-1 rides the gpsimd SWDGE ring, later items
            # the sync ring; out stores go on scalar. ---
            at_sp = cpool.tile([P, kt, npr], F8, tag="at_sp")
            at_tm = cpool.tile([P, kt, npr], F8, tag="at_tm")
            nh = npr // 2
            for k in range(kt):
                # split every tile across both HWDGE rings so tile k
                # completes early, tracking PE consumption
                nc.sync.dma_start(at_sp[:, k, :nh], atsp_d[:, k, :nh])
                nc.scalar.dma_start(at_sp[:, k, nh:], atsp_d[:, k, nh:])

            w_sb = []
            for i, wd in enumerate(w_d):
                w = cpool.tile([P, ct, c], F8, tag=f"w{i}")
                nc.scalar.dma_start(w[:], wd[:])
                w_sb.append(w)
            b1_sb = cpool.tile([P, ct], F32, tag="b1")
            b2_sb = cpool.tile([P, ct], F32, tag="b2")
            nc.scalar.dma_start(b1_sb[:], b1_d[:])
            nc.scalar.dma_start(b2_sb[:], b2_d[:])

            def emit_load_at_tm():
                # queued on the rings behind at_sp (and behind item-1's x on
                # sync) -- needed only from item-0 phase 4
                for k in range(kt):
                    nc.sync.dma_start(at_tm[:, k, :nh], attm_d[:, k, :nh])
                    nc.scalar.dma_start(at_tm[:, k, nh:], attm_d[:, k, nh:])

            bias_tile = n // P      # global node index n == first pad row
            bias_part = n % P

            def emit_load_x8(b):
                # fp8 x (phase-1 stationary) on the gpsimd SWDGE ring; it is
                # otherwise idle and x8 is needed early. Pad rows zeroed.
                x8 = xqp.tile([P, kt, c], F8, tag="xq", name=f"x8_{b}")
                for k in range(kt):
                    rows = min(P, n - k * P)
                    if rows < P:
                        nc.vector.memset(x8[:, k, :], 0)
                    if rows > 0:
                        nc.gpsimd.dma_start(x8[:rows, k, :],
                                            x8_d[b, k * P:k * P + rows, :])
                return x8

            def emit_load_xbf(b):
                # bf16 residual, needed only at phase 6: items 0-1 ride
                # gpsimd behind the x8 pair, later items the sync ring
                # (which frees up after at_tm)
                x_eng = nc.gpsimd if b <= 1 else nc.sync
                xbf = xbfp.tile([P, kt, c], BF16, tag="xbf", name=f"xbf_{b}")
                for k in range(kt):
                    rows = min(P, n - k * P)
                    if rows > 0:
                        x_eng.dma_start(xbf[:rows, k, :],
                                        x_d[b, k * P:k * P + rows, :])
                return xbf

            def emit_p1(b, x8):
                # phase 1: g1T = (A_sp @ x)^T; DoubleRow over k-tile pairs.
                # Pair-outer per cc over 4 parallel PSUM banks: one ldweights
                # per (cc, pair) serves all 4 quarters, and item-0 consumes
                # each at_sp tile pair as soon as its DMAs land.
                g1T = actp.tile([P, ct, npad], F8, tag="act", name=f"g1T_{b}")
                for cc in range(ct):
                    groups = [(psA.tile([P, 512], F32, tag="psA",
                                        name=f"ps1_{b}_{cc}_{q0}"), q0, qs)
                              for (q0, qs) in nqv]
                    for k in range(kp):
                        for (ps, q0, qs) in groups:
                            nc.tensor.matmul(
                                ps[:, :qs],
                                lhsT=x8[:, 2 * k:2 * k + 2,
                                        cc * P:(cc + 1) * P],
                                rhs=at_sp[:, 2 * k:2 * k + 2, q0:q0 + qs],
                                start=(k == 0), stop=(k == kp - 1),
                                perf_mode=DR)
                    for (ps, q0, qs) in groups:
                        nc.vector.tensor_copy(g1T[:, cc, q0:q0 + qs],
                                              ps[:, :qs])
                return g1T

            def emit_p2(b, g1T):
                # phase 2: a1T = relu(W1^T @ g1T + b1); single DoubleRow
                # instruction contracts both ci tiles
                a1T = actp.tile([P, ct, npad], F8, tag="act", name=f"a1T_{b}")
                for cc in range(ct):
                    # cols [n:npad] are read as phase-3 lhsT pads but never
                    # written by the trimmed quarters
                    nc.vector.memset(a1T[:, cc, n:npad], 0)
                for co in range(ct):
                    for (q0, qs) in nqv:
                        ps = psA.tile([P, 512], F32, tag="psA")
                        nc.tensor.matmul(
                            ps[:, :qs],
                            lhsT=w_sb[0][:, 0:2, co * P:(co + 1) * P],
                            rhs=g1T[:, 0:2, q0:q0 + qs],
                            start=True, stop=True, perf_mode=DR)
                        nc.scalar.activation(a1T[:, co, q0:q0 + qs], ps[:, :qs],
                                             RELU, bias=b1_sb[:, co:co + 1])
                return a1T

            def emit_p3(b, a1T):
                # phase 3: h2 = a1 @ W2 (natural layout)
                h2 = hp.tile([P, kt, c], F8, tag="h", name=f"h2_{b}")
                for k in range(kt):
                    ps = psW.tile([P, c], F32, tag="psW")
                    nc.tensor.matmul(
                        ps[:],
                        lhsT=a1T[:, 0:2, k * P:(k + 1) * P],
                        rhs=w_sb[1][:, 0:2, :],
                        start=True, stop=True, perf_mode=DR)
                    # alternate drains across DVE/Act so the copy chain
                    # keeps pace with the 256-col matmuls
                    if k % 2 == 0:
                        nc.vector.tensor_copy(h2[:, k, :], ps[:])
                    else:
                        nc.scalar.copy(h2[:, k, :], ps[:])
                return h2

            def emit_p4(b, h2):
                # phase 4: a2T = relu((A_tm @ h2)^T + b2); pair-outer per cc
                # (4-bank rounds) so item-0 consumes at_tm pairs as they land
                a2T = actp.tile([P, ct, npad], F8, tag="act", name=f"a2T_{b}")
                for cc in range(ct):
                    nc.vector.memset(a2T[:, cc, n:npad], 0)
                for cc in range(ct):
                    groups = [(psA.tile([P, 512], F32, tag="psA",
                                        name=f"ps4_{b}_{cc}_{q0}"), q0, qs)
                              for (q0, qs) in nqv]
                    for k in range(kp):
                        for (ps, q0, qs) in groups:
                            nc.tensor.matmul(
                                ps[:, :qs],
                                lhsT=h2[:, 2 * k:2 * k + 2,
                                        cc * P:(cc + 1) * P],
                                rhs=at_tm[:, 2 * k:2 * k + 2, q0:q0 + qs],
                                start=(k == 0), stop=(k == kp - 1),
                                perf_mode=DR)
                    for (ps, q0, qs) in groups:
                        nc.scalar.activation(a2T[:, cc, q0:q0 + qs],
                                             ps[:, :qs], RELU,
                                             bias=b2_sb[:, cc:cc + 1])
                return a2T

            def emit_p5(b, a2T):
                # phase 5: h3 = a2 @ W3; h3[row n] = b3
                h3 = hp.tile([P, kt, c], F8, tag="h", name=f"h3_{b}")
                for k in range(kt):
                    ps = psW.tile([P, c], F32, tag="psW")
                    nc.tensor.matmul(
                        ps[:],
                        lhsT=a2T[:, 0:2, k * P:(k + 1) * P],
                        rhs=w_sb[2][:, 0:2, :],
                        start=True, stop=True, perf_mode=DR)
                    if k % 2 == 0:
                        nc.vector.tensor_copy(h3[:, k, :], ps[:])
                    else:
                        nc.scalar.copy(h3[:, k, :], ps[:])
                nc.scalar.dma_start(
                    h3[bias_part:bias_part + 1, bias_tile, :], b3_d[:, :])
                return h3

            def emit_p6(b, xbf, h3):
                # phase 6: out = relu(A_sp @ h3 + x), residual from the
                # resident bf16 x tile
                for ko in range(kt):
                    rows = min(P, n - ko * P)
                    if rows <= 0:
                        continue
                    ps = psW.tile([P, c], F32, tag="psW")
                    for k in range(kp):
                        nc.tensor.matmul(
                            ps[:rows, :],
                            lhsT=at_sp[:, 2 * k:2 * k + 2,
                                       ko * P:ko * P + rows],
                            rhs=h3[:, 2 * k:2 * k + 2, :],
                            start=(k == 0), stop=(k == kp - 1),
                            perf_mode=DR)
                    ot = outp.tile([P, c], BF16, tag="o")
                    nc.vector.tensor_add(ot[:rows, :], ps[:rows, :],
                                         xbf[:rows, ko, :])
                    nc.scalar.activation(ot[:rows, :], ot[:rows, :], RELU)
                    st_eng = nc.sync if ko % 2 == 0 else nc.scalar
                    st_eng.dma_start(out_d[b, ko * P:ko * P + rows, :],
                                     ot[:rows, :])

            def emit_p5_pair(b, a2T, h3p, ip):
                # phase 5 variant writing into flat slot ip of a 2-item h3
                for k in range(kt):
                    ps = psW.tile([P, c], F32, tag="psW")
                    nc.tensor.matmul(
                        ps[:],
                        lhsT=a2T[:, 0:2, k * P:(k + 1) * P],
                        rhs=w_sb[2][:, 0:2, :],
                        start=True, stop=True, perf_mode=DR)
                    if k % 2 == 0:
                        nc.vector.tensor_copy(h3p[:, k, ip * c:(ip + 1) * c],
                                              ps[:])
                    else:
                        nc.scalar.copy(h3p[:, k, ip * c:(ip + 1) * c], ps[:])
                nc.scalar.dma_start(
                    h3p[bias_part:bias_part + 1, bias_tile,
                        ip * c:(ip + 1) * c], b3_d[:, :])

            def emit_p6_pair(b0, xbfs, h3p):
                # phase 6 over two items: flat F=512 single-pass matmuls
                # sharing the AT_sp stationary operand across the pair
                for ko in range(kt):
                    rows = min(P, n - ko * P)
                    if rows <= 0:
                        continue
                    ps = psW.tile([P, 2 * c], F32, tag="psW")
                    for k in range(kp):
                        nc.tensor.matmul(
                            ps[:rows, :],
                            lhsT=at_sp[:, 2 * k:2 * k + 2,
                                       ko * P:ko * P + rows],
                            rhs=h3p[:, 2 * k:2 * k + 2, :],
                            start=(k == 0), stop=(k == kp - 1),
                            perf_mode=DR)
                    for ip in range(2):
                        ot = outp.tile([P, c], BF16, tag="o")
                        nc.vector.tensor_add(ot[:rows, :],
                                             ps[:rows, ip * c:(ip + 1) * c],
                                             xbfs[ip][:rows, ko, :])
                        nc.scalar.activation(ot[:rows, :], ot[:rows, :], RELU)
                        st_eng = nc.sync if (ko + ip) % 2 == 0 else nc.scalar
                        st_eng.dma_start(
                            out_d[b0 + ip, ko * P:ko * P + rows, :],
                            ot[:rows, :])

            def emit_mid(b, g1T, h3p, ip):
                a1T = emit_p2(b, g1T)
                h2 = emit_p3(b, a1T)
                a2T = emit_p4(b, h2)
                emit_p5_pair(b, a2T, h3p, ip)

            def emit_pair(b0, xbf0=None, g1T0=None, xbf1=None, g1T1=None):
                if xbf0 is None:
                    x80 = emit_load_x8(b0)
                    xbf0 = emit_load_xbf(b0)
                    g1T0 = emit_p1(b0, x80)
                if xbf1 is None:
                    x81 = emit_load_x8(b0 + 1)
                    xbf1 = emit_load_xbf(b0 + 1)
                else:
                    x81 = None
                h3p = hpp.tile([P, kt, 2 * c], F8, tag="hpair",
                               name=f"h3p_{b0}")
                emit_mid(b0, g1T0, h3p, 0)
                if g1T1 is None:
                    g1T1 = emit_p1(b0 + 1, x81)
                emit_mid(b0 + 1, g1T1, h3p, 1)
                emit_p6_pair(b0, [xbf0, xbf1], h3p)

            def emit_item(b, xbf=None, g1T=None):
                if xbf is None:
                    x8 = emit_load_x8(b)
                    xbf = emit_load_xbf(b)
                    g1T = emit_p1(b, x8)
                a1T = emit_p2(b, g1T)
                h2 = emit_p3(b, a1T)
                a2T = emit_p4(b, h2)
                h3 = emit_p5(b, a2T)
                emit_p6(b, xbf, h3)

            # Emission order: item-1 phase 1 is hoisted between item-0
            # phase 1 and phase 2 so the PE has more matmul work queued
            # before the first at_tm use (its DMA trails at_sp). The x8
            # pair loads front-run both xbf loads on the gpsimd ring.
            x80 = emit_load_x8(0)
            if bl > 1:
                x81 = emit_load_x8(1)
            g1T0 = emit_p1(0, x80)
            xbf0 = emit_load_xbf(0)
            if bl > 1:
                xbf1 = emit_load_xbf(1)
                emit_load_at_tm()
                g1T1 = emit_p1(1, x81)
            else:
                emit_load_at_tm()
            if bl > 1 and bl % 2 == 0:
                emit_pair(0, xbf0, g1T0, xbf1, g1T1)
                for b0 in range(2, bl, 2):
                    emit_pair(b0)
            else:
                emit_item(0, xbf0, g1T0)
                if bl > 1:
                    emit_item(1, xbf1, g1T1)
                for b in range(2, bl):
                    emit_item(b)

    nc.compile()
    return nc


def _norm_adj_T(edges, n, npad, bias_row):
    """A^T padded to [npad, npad] in fp32. AT[m, j] = A[j, m] where
    out[j] += A[j, m] * h[m]; edge (r -> c) contributes dinv[r]*dinv[c] at
    AT[r, c]. Self loops included. If bias_row, AT[n, :n] = 1 (bias fold)."""
    row = np.concatenate([edges[0], np.arange(n, dtype=np.int64)])
    col = np.concatenate([edges[1], np.arange(n, dtype=np.int64)])
    deg = np.bincount(col, minlength=n).astype(np.float32)
    dinv = np.zeros(n, np.float32)
    nz = deg > 0
    dinv[nz] = 1.0 / np.sqrt(deg[nz])
    norm = dinv[row] * dinv[col]
    at = np.zeros((npad, npad), np.float32)
    np.add.at(at, (row, col), norm)
    if bias_row:
        at[n, :n] = 1.0
    return at


def _tile_rows(a, kt):
    """[kt*P, F] -> [P, kt, F] so that [p, k, :] = a[k*P + p, :]."""
    return np.ascontiguousarray(
        a.reshape(kt, P, a.shape[-1]).transpose(1, 0, 2))


_PROGRAM_CACHE = {}


def _get_program(bl, n, c):
    key = (bl, n, c)
    if key not in _PROGRAM_CACHE:
        _PROGRAM_CACHE[key] = build_program(bl, n, c)
    return _PROGRAM_CACHE[key]


def run(inputs, trace=False, n_cores=N_CORES):
    x32 = np.asarray(inputs["x"], dtype=np.float32)
    x = x32.astype(NP_BF16)
    x8 = x32.astype(NP_F8)
    w1 = np.asarray(inputs["W1"], np.float32)
    w2 = np.asarray(inputs["W2"], np.float32)
    w3 = np.asarray(inputs["W3"], np.float32)
    b1 = np.asarray(inputs["b1"], np.float32)
    b2 = np.asarray(inputs["b2"], np.float32)
    b3 = np.asarray(inputs["b3"], np.float32)
    e_sp = np.asarray(inputs["keypoint_line_without_temporal"]).astype(np.int64)
    e_tm = np.asarray(inputs["keypoint_line_with_temporal"]).astype(np.int64)

    b_total, n, c = x.shape
    bl = b_total // n_cores
    kt = -(-(n + 1) // P)
    npad = kt * P
    ct = c // P

    nc = _get_program(bl, n, c)

    npr = -(-n // 16) * 16
    at_sp = _tile_rows(
        _norm_adj_T(e_sp, n, npad, bias_row=True)[:, :npr].astype(NP_F8), kt)
    at_tm = _tile_rows(
        _norm_adj_T(e_tm, n, npad, bias_row=False)[:, :npr].astype(NP_F8), kt)
    shared = {
        "at_sp": at_sp,
        "at_tm": at_tm,
        "w1": _tile_rows(w1.astype(NP_F8), ct),
        "w2": _tile_rows(w2.astype(NP_F8), ct),
        "w3": _tile_rows(w3.astype(NP_F8), ct),
        "b1": np.ascontiguousarray(b1.reshape(ct, P).T),
        "b2": np.ascontiguousarray(b2.reshape(ct, P).T),
        "b3": np.ascontiguousarray(b3.astype(NP_F8)[None, :]),
    }
    in_maps = [
        {"x": np.ascontiguousarray(x[i * bl:(i + 1) * bl]),
         "x8": np.ascontiguousarray(x8[i * bl:(i + 1) * bl]), **shared}
        for i in range(n_cores)
    ]
    res = run_bass_kernel_spmd(nc, in_maps, core_ids=list(range(n_cores)),
                               trace=trace)
    out = np.concatenate(
        [np.asarray(r["out"]).astype(np.float32) for r in res.results], axis=0)
    return out, res


def kernel(**inputs) -> np.ndarray:
    out, _ = run(inputs, trace=False)
    return out


# revision 21
# speedup vs baseline: 1.6845x; 1.0383x over previous
"""Trainium2 Bass kernel for nn_BasicBlock (3-layer GCN block with residual).

Math (per batch item b, per conv):
    out = A @ (x @ W) + bias,  A = normalized adjacency (with self loops)
where A[c, r] = sum over edges r->c of dinv[r]*dinv[c] (dense N x N, shared
across batch and precomputed on host from the edge lists).

Block:
    a1 = relu(A_sp @ (x  @ W1) + b1)
    a2 = relu(A_tm @ (a1 @ W2) + b2)
    o3 =      A_sp @ (a2 @ W3) + b3
    out = relu(o3 + x)

On-chip layouts per item (P=128 partitions):
    natural  [n, c] : node chunks on partitions           (rhs of A-matmul /
                                                           lhsT of form-iv)
    transposed [c, n]: channel chunks on partitions        (consumed by W-matmul)

Phases per item (matmul forms; AT = A^T so AT[m, n] = A[n, m]):
    1. g1T[c,n]  = sum_m x[m,c]  * AT_sp[m,n]      (lhsT=x chunk,  rhs=AT_sp)
    2. a1T[co,n] = relu(sum_ci W1[ci,co]*g1T[ci,n] + b1)   (lhsT=W1, rhs=g1T)
    3. h2[n,c]   = sum_ci a1T[ci,n] * W2[ci,c]     (lhsT=a1T chunk, rhs=W2)
    4. a2T[c,n]  = relu(sum_m h2[m,c]*AT_tm[m,n] + b2)
    5. h3[n,c]   = sum_ci a2T[ci,n] * W3[ci,c];  h3[N,:] = b3
    6. out[n,c]  = relu(sum_m AT_sp[m,n]*h3[m,c] + x[n,c])
       (AT_sp row N is all-ones over valid cols -> adds b3 to every node;
        harmless in phase 1 because x row N is zero-padded)

All matmuls run in fp8 e4m3 with DoubleRow perf mode (two 128-deep k-tiles
contracted per instruction) and fp32 PSUM accumulation. The conv-path signal
is tiny relative to the identity residual, so fp8 error washes out; the
residual itself stays bf16 (x arrives from host in both bf16 and fp8).
Batch (64) is sharded 8 items/core over the 8 cores; A/W/b are replicated.
"""

import sys

if "/opt/trn_rl_repo" not in sys.path:
    sys.path.insert(0, "/opt/trn_rl_repo")

import numpy as np
import ml_dtypes

import concourse.bass as bass
import concourse.bacc as bacc
import concourse.mybir as mybir
import concourse.tile as tile
from concourse.bass_utils import run_bass_kernel_spmd

P = 128
B, N, C = 64, 1700, 256
N_CORES = 8
B_LOCAL = B // N_CORES

F32 = mybir.dt.float32
BF16 = mybir.dt.bfloat16
F8 = mybir.dt.float8e4
RELU = mybir.ActivationFunctionType.Relu
DR = mybir.MatmulPerfMode.DoubleRow
NP_BF16 = ml_dtypes.bfloat16
NP_F8 = ml_dtypes.float8_e4m3


def _quarters(total, step=512):
    return [(q, min(step, total - q)) for q in range(0, total, step)]


def build_program(bl, n, c):
    """Build the Bass/Tile program for `bl` batch items, `n` nodes, `c` chans."""
    kt = -(-(n + 1) // P)  # node chunks; >= one pad row (bias row at index n)
    assert kt % 2 == 0, "DoubleRow pairing needs an even k-tile count"
    kp = kt // 2
    npad = kt * P
    npr = -(-n // 16) * 16  # at-tile row pitch: DoubleRow needs 16B-aligned strides
    ct = c // P
    cp = ct // 2
    nq = _quarters(npad)

    nqv = _quarters(n)  # valid-column quarters (phases whose pads are unread)

    nc = bacc.Bacc("TRN2", target_bir_lowering=False, debug=False,
                   enable_asserts=False)

    x8_d = nc.dram_tensor("x8", [bl, npad, c], F8, kind="ExternalInput")
    x_d = nc.dram_tensor("x", [bl, n, c], BF16, kind="ExternalInput")
    atsp_d = nc.dram_tensor("at_sp", [P, kt, npr], F8, kind="ExternalInput")
    attm_d = nc.dram_tensor("at_tm", [P, kt, npr], F8, kind="ExternalInput")
    w_d = [nc.dram_tensor(f"w{i}", [P, ct, c], F8, kind="ExternalInput")
           for i in (1, 2, 3)]
    b1_d = nc.dram_tensor("b1", [P, ct], F32, kind="ExternalInput")
    b2_d = nc.dram_tensor("b2", [P, ct], F32, kind="ExternalInput")
    b3_d = nc.dram_tensor("b3", [1, c], F8, kind="ExternalInput")
    id_d = nc.dram_tensor("ident", [P, P], BF16, kind="ExternalInput")
    out_d = nc.dram_tensor("out", [bl, n, c], BF16, kind="ExternalOutput")

    with tile.TileContext(nc) as tc:
        with (
            tc.tile_pool(name="const", bufs=1) as cpool,
            tc.tile_pool(name="xq", bufs=5) as xqp,
            tc.tile_pool(name="xbf", bufs=5) as xbfp,
            tc.tile_pool(name="act", bufs=5) as actp,
            tc.tile_pool(name="h", bufs=3) as hp,
            tc.tile_pool(name="hpair", bufs=1) as hpp,
            tc.tile_pool(name="outp", bufs=4) as outp,
            tc.tile_pool(name="psA", bufs=4, space="PSUM") as psA,
            tc.tile_pool(name="psW", bufs=4, space="PSUM") as psW,
        ):
            # --- constants.  Ring plan: x8_0 front-runs everything on the
            # two fast HWDGE rings (it gates the first matmuls), then at_sp
            # in thirds (sync/scalar/vector), then w+b on scalar, then at_tm
            # halves behind those; the remaining x rides gpsimd / late sync;
            # out stores go on sync. ---
            at_sp = cpool.tile([P, kt, npr], F8, tag="at_sp")
            at_tm = cpool.tile([P, kt, npr], F8, tag="at_tm")
            w_sb = [cpool.tile([P, ct, c], F8, tag=f"w{i}", name=f"w{i}")
                    for i in range(3)]
            b1_sb = cpool.tile([P, ct], F32, tag="b1")
            b2_sb = cpool.tile([P, ct], F32, tag="b2")
            id_sb = cpool.tile([P, P], BF16, tag="ident")
            nh = npr // 2

            def emit_load_at_sp():
                for k in range(kt):
                    # split every tile across both HWDGE rings so tile k
                    # completes ahead of PE consumption
                    nc.sync.dma_start(at_sp[:, k, :nh], atsp_d[:, k, :nh])
                    nc.scalar.dma_start(at_sp[:, k, nh:], atsp_d[:, k, nh:])

            def emit_load_w_b():
                for w, wd in zip(w_sb, w_d):
                    nc.scalar.dma_start(w[:], wd[:])
                nc.scalar.dma_start(b1_sb[:], b1_d[:])
                nc.scalar.dma_start(b2_sb[:], b2_d[:])
                nc.scalar.dma_start(id_sb[:], id_d[:])

            def emit_load_at_tm():
                # queued on the rings behind at_sp (and behind item-1's x on
                # sync) -- needed only from item-0 phase 4
                for k in range(kt):
                    nc.sync.dma_start(at_tm[:, k, :nh], attm_d[:, k, :nh])
                    nc.scalar.dma_start(at_tm[:, k, nh:], attm_d[:, k, nh:])

            bias_tile = n // P      # global node index n == first pad row
            bias_part = n % P

            def emit_load_x8(b):
                # fp8 x (phase-1 stationary), host-padded to npad rows so the
                # whole tile DMAs without memsets. Item 0 gates the very first
                # matmuls, so it is split across the two fast HWDGE rings
                # ahead of at_sp; later items ride the idle gpsimd SWDGE ring.
                x8 = xqp.tile([P, kt, c], F8, tag="xq", name=f"x8_{b}")
                for k in range(kt):
                    if b == 0 and k < 2:
                        # only the first pair front-runs at_sp on the fast
                        # rings (early descriptors cost ~0.6us each); the
                        # rest leads the gpsimd queue and stays ahead of
                        # the at_sp-paced rounds
                        eng = nc.sync if k % 2 == 0 else nc.scalar
                    else:
                        eng = nc.gpsimd
                    eng.dma_start(x8[:, k, :], x8_d[b, k * P:(k + 1) * P, :])
                return x8

            def emit_load_xbf(b):
                # bf16 residual, needed only at phase 6: items 0-1 ride
                # gpsimd behind the x8 pair, later items the sync ring
                # (which frees up after at_tm)
                x_eng = nc.gpsimd if b <= 1 else nc.sync
                xbf = xbfp.tile([P, kt, c], BF16, tag="xbf", name=f"xbf_{b}")
                for k in range(kt):
                    rows = min(P, n - k * P)
                    if rows > 0:
                        x_eng.dma_start(xbf[:rows, k, :],
                                        x_d[b, k * P:k * P + rows, :])
                return xbf

            def emit_p1(b, x8):
                # phase 1: g1T = (A_sp @ x)^T; DoubleRow over k-tile pairs.
                # Pair-outer per cc over 4 parallel PSUM banks: one ldweights
                # per (cc, pair) serves all 4 quarters, and item-0 consumes
                # each at_sp tile pair as soon as its DMAs land.
                g1T = actp.tile([P, ct, npad], F8, tag="act", name=f"g1T_{b}")
                for cc in range(ct):
                    groups = [(psA.tile([P, 512], F32, tag="psA",
                                        name=f"ps1_{b}_{cc}_{q0}"), q0, qs)
                              for (q0, qs) in nqv]
                    for k in range(kp):
                        for (ps, q0, qs) in groups:
                            nc.tensor.matmul(
                                ps[:, :qs],
                                lhsT=x8[:, 2 * k:2 * k + 2,
                                        cc * P:(cc + 1) * P],
                                rhs=at_sp[:, 2 * k:2 * k + 2, q0:q0 + qs],
                                start=(k == 0), stop=(k == kp - 1),
                                perf_mode=DR)
                    for (ps, q0, qs) in groups:
                        nc.vector.tensor_copy(g1T[:, cc, q0:q0 + qs],
                                              ps[:, :qs])
                return g1T

            def emit_p2(b, g1T):
                # phase 2: a1T = relu(W1^T @ g1T + b1); single DoubleRow
                # instruction contracts both ci tiles
                a1T = actp.tile([P, ct, npad], F8, tag="act", name=f"a1T_{b}")
                for cc in range(ct):
                    # cols [n:npad] are read as phase-3 lhsT pads but never
                    # written by the trimmed quarters
                    nc.vector.memset(a1T[:, cc, n:npad], 0)
                for co in range(ct):
                    for (q0, qs) in nqv:
                        ps = psA.tile([P, 512], F32, tag="psA")
                        nc.tensor.matmul(
                            ps[:, :qs],
                            lhsT=w_sb[0][:, 0:2, co * P:(co + 1) * P],
                            rhs=g1T[:, 0:2, q0:q0 + qs],
                            start=True, stop=True, perf_mode=DR)
                        nc.scalar.activation(a1T[:, co, q0:q0 + qs], ps[:, :qs],
                                             RELU, bias=b1_sb[:, co:co + 1])
                return a1T

            def emit_p3(b, a1T):
                # phase 3: h2 = a1 @ W2 (natural layout)
                h2 = hp.tile([P, kt, c], F8, tag="h", name=f"h2_{b}")
                for k in range(kt):
                    ps = psW.tile([P, c], F32, tag="psW")
                    nc.tensor.matmul(
                        ps[:],
                        lhsT=a1T[:, 0:2, k * P:(k + 1) * P],
                        rhs=w_sb[1][:, 0:2, :],
                        start=True, stop=True, perf_mode=DR)
                    # alternate drains across DVE/Act so the copy chain
                    # keeps pace with the 256-col matmuls
                    if k % 2 == 0:
                        nc.vector.tensor_copy(h2[:, k, :], ps[:])
                    else:
                        nc.scalar.copy(h2[:, k, :], ps[:])
                return h2

            def emit_p4(b, h2):
                # phase 4: a2T = relu((A_tm @ h2)^T + b2); pair-outer per cc
                # (4-bank rounds) so item-0 consumes at_tm pairs as they land
                a2T = actp.tile([P, ct, npad], F8, tag="act", name=f"a2T_{b}")
                for cc in range(ct):
                    nc.vector.memset(a2T[:, cc, n:npad], 0)
                for cc in range(ct):
                    groups = [(psA.tile([P, 512], F32, tag="psA",
                                        name=f"ps4_{b}_{cc}_{q0}"), q0, qs)
                              for (q0, qs) in nqv]
                    for k in range(kp):
                        for (ps, q0, qs) in groups:
                            nc.tensor.matmul(
                                ps[:, :qs],
                                lhsT=h2[:, 2 * k:2 * k + 2,
                                        cc * P:(cc + 1) * P],
                                rhs=at_tm[:, 2 * k:2 * k + 2, q0:q0 + qs],
                                start=(k == 0), stop=(k == kp - 1),
                                perf_mode=DR)
                    for (ps, q0, qs) in groups:
                        nc.scalar.activation(a2T[:, cc, q0:q0 + qs],
                                             ps[:, :qs], RELU,
                                             bias=b2_sb[:, cc:cc + 1])
                return a2T

            def emit_p5(b, a2T):
                # phase 5: h3 = a2 @ W3; h3[row n] = b3
                h3 = hp.tile([P, kt, c], F8, tag="h", name=f"h3_{b}")
                for k in range(kt):
                    ps = psW.tile([P, c], F32, tag="psW")
                    nc.tensor.matmul(
                        ps[:],
                        lhsT=a2T[:, 0:2, k * P:(k + 1) * P],
                        rhs=w_sb[2][:, 0:2, :],
                        start=True, stop=True, perf_mode=DR)
                    if k % 2 == 0:
                        nc.vector.tensor_copy(h3[:, k, :], ps[:])
                    else:
                        nc.scalar.copy(h3[:, k, :], ps[:])
                nc.scalar.dma_start(
                    h3[bias_part:bias_part + 1, bias_tile, :], b3_d[:, :])
                return h3

            def emit_p6(b, xbf, h3):
                # phase 6: out = relu(A_sp @ h3 + x), residual from the
                # resident bf16 x tile
                for ko in range(kt):
                    rows = min(P, n - ko * P)
                    if rows <= 0:
                        continue
                    ps = psW.tile([P, c], F32, tag="psW")
                    for k in range(kp):
                        nc.tensor.matmul(
                            ps[:rows, :],
                            lhsT=at_sp[:, 2 * k:2 * k + 2,
                                       ko * P:ko * P + rows],
                            rhs=h3[:, 2 * k:2 * k + 2, :],
                            start=(k == 0), stop=(k == kp - 1),
                            perf_mode=DR)
                    ot = outp.tile([P, c], BF16, tag="o")
                    nc.vector.tensor_add(ot[:rows, :], ps[:rows, :],
                                         xbf[:rows, ko, :])
                    nc.scalar.activation(ot[:rows, :], ot[:rows, :], RELU)
                    nc.sync.dma_start(out_d[b, ko * P:ko * P + rows, :],
                                      ot[:rows, :])

            def emit_p5_pair(b, a2T, h3p, ip):
                # phase 5 variant writing into flat slot ip of a 2-item h3
                for k in range(kt):
                    ps = psW.tile([P, c], F32, tag="psW")
                    nc.tensor.matmul(
                        ps[:],
                        lhsT=a2T[:, 0:2, k * P:(k + 1) * P],
                        rhs=w_sb[2][:, 0:2, :],
                        start=True, stop=True, perf_mode=DR)
                    if k % 2 == 0:
                        nc.vector.tensor_copy(h3p[:, k, ip * c:(ip + 1) * c],
                                              ps[:])
                    else:
                        nc.scalar.copy(h3p[:, k, ip * c:(ip + 1) * c], ps[:])
                nc.scalar.dma_start(
                    h3p[bias_part:bias_part + 1, bias_tile,
                        ip * c:(ip + 1) * c], b3_d[:, :])

            def emit_p6_pair(b0, xbfs, h3p, last=False):
                # phase 6 over two items: flat F=512 single-pass matmuls
                # sharing the AT_sp stationary operand across the pair. For
                # the final pair the residual is accumulated on the PE via
                # an identity matmul so the drain (which nothing overlaps)
                # is just relu+store instead of add+relu+store.
                for ko in range(kt):
                    rows = min(P, n - ko * P)
                    if rows <= 0:
                        continue
                    ps = psW.tile([P, 2 * c], F32, tag="psW")
                    for k in range(kp):
                        nc.tensor.matmul(
                            ps[:rows, :],
                            lhsT=at_sp[:, 2 * k:2 * k + 2,
                                       ko * P:ko * P + rows],
                            rhs=h3p[:, 2 * k:2 * k + 2, :],
                            start=(k == 0), stop=(k == kp - 1 and not last),
                            perf_mode=DR)
                    if last:
                        for ip in range(2):
                            # contract over only `rows` partitions: the xbf
                            # pad rows are uninitialized and NaN*0 = NaN
                            nc.tensor.matmul(
                                ps[:rows, ip * c:(ip + 1) * c],
                                lhsT=id_sb[:rows, :rows],
                                rhs=xbfs[ip][:rows, ko, :],
                                start=False, stop=(ip == 1))
                    for ip in range(2):
                        ot = outp.tile([P, c], BF16, tag="o")
                        if last:
                            nc.scalar.activation(
                                ot[:rows, :], ps[:rows, ip * c:(ip + 1) * c],
                                RELU)
                        else:
                            nc.vector.tensor_add(
                                ot[:rows, :], ps[:rows, ip * c:(ip + 1) * c],
                                xbfs[ip][:rows, ko, :])
                            nc.scalar.activation(ot[:rows, :], ot[:rows, :],
                                                 RELU)
                        nc.sync.dma_start(
                            out_d[b0 + ip, ko * P:ko * P + rows, :],
                            ot[:rows, :])

            def emit_mid(b, g1T, h3p, ip):
                a1T = emit_p2(b, g1T)
                h2 = emit_p3(b, a1T)
                a2T = emit_p4(b, h2)
                emit_p5_pair(b, a2T, h3p, ip)

            def emit_pair(b0, xbf0=None, g1T0=None, xbf1=None, g1T1=None):
                if xbf0 is None:
                    x80 = emit_load_x8(b0)
                    xbf0 = emit_load_xbf(b0)
                    g1T0 = emit_p1(b0, x80)
                if xbf1 is None:
                    x81 = emit_load_x8(b0 + 1)
                    xbf1 = emit_load_xbf(b0 + 1)
                else:
                    x81 = None
                h3p = hpp.tile([P, kt, 2 * c], F8, tag="hpair",
                               name=f"h3p_{b0}")
                emit_mid(b0, g1T0, h3p, 0)
                if g1T1 is None:
                    g1T1 = emit_p1(b0 + 1, x81)
                emit_mid(b0 + 1, g1T1, h3p, 1)
                emit_p6_pair(b0, [xbf0, xbf1], h3p, last=(b0 == bl - 2))

            def emit_item(b, xbf=None, g1T=None):
                if xbf is None:
                    x8 = emit_load_x8(b)
                    xbf = emit_load_xbf(b)
                    g1T = emit_p1(b, x8)
                a1T = emit_p2(b, g1T)
                h2 = emit_p3(b, a1T)
                a2T = emit_p4(b, h2)
                h3 = emit_p5(b, a2T)
                emit_p6(b, xbf, h3)

            # Emission order: x8_0 chunks enqueue ahead of at_sp on the
            # fast rings; item-1 phase 1 is hoisted between item-0 phase 1
            # and phase 2 so the PE has more matmul work queued before the
            # first at_tm use (its DMA trails at_sp).
            x80 = emit_load_x8(0)
            emit_load_at_sp()
            emit_load_w_b()
            if bl > 1:
                x81 = emit_load_x8(1)
            g1T0 = emit_p1(0, x80)
            xbf0 = emit_load_xbf(0)
            if bl > 1:
                xbf1 = emit_load_xbf(1)
                emit_load_at_tm()
                g1T1 = emit_p1(1, x81)
            else:
                emit_load_at_tm()
            if bl > 1 and bl % 2 == 0:
                emit_pair(0, xbf0, g1T0, xbf1, g1T1)
                for b0 in range(2, bl, 2):
                    emit_pair(b0)
            else:
                emit_item(0, xbf0, g1T0)
                if bl > 1:
                    emit_item(1, xbf1, g1T1)
                for b in range(2, bl):
                    emit_item(b)

    nc.compile()
    return nc


def _norm_adj_T(edges, n, npad, bias_row):
    """A^T padded to [npad, npad] in fp32. AT[m, j] = A[j, m] where
    out[j] += A[j, m] * h[m]; edge (r -> c) contributes dinv[r]*dinv[c] at
    AT[r, c]. Self loops included. If bias_row, AT[n, :n] = 1 (bias fold)."""
    row = np.concatenate([edges[0], np.arange(n, dtype=np.int64)])
    col = np.concatenate([edges[1], np.arange(n, dtype=np.int64)])
    deg = np.bincount(col, minlength=n).astype(np.float32)
    dinv = np.zeros(n, np.float32)
    nz = deg > 0
    dinv[nz] = 1.0 / np.sqrt(deg[nz])
    norm = dinv[row] * dinv[col]
    at = np.zeros((npad, npad), np.float32)
    np.add.at(at, (row, col), norm)
    if bias_row:
        at[n, :n] = 1.0
    return at


def _tile_rows(a, kt):
    """[kt*P, F] -> [P, kt, F] so that [p, k, :] = a[k*P + p, :]."""
    return np.ascontiguousarray(
        a.reshape(kt, P, a.shape[-1]).transpose(1, 0, 2))


_PROGRAM_CACHE = {}


def _get_program(bl, n, c):
    key = (bl, n, c)
    if key not in _PROGRAM_CACHE:
        _PROGRAM_CACHE[key] = build_program(bl, n, c)
    return _PROGRAM_CACHE[key]


def run(inputs, trace=False, n_cores=N_CORES):
    x32 = np.asarray(inputs["x"], dtype=np.float32)
    x = x32.astype(NP_BF16)
    npad_h = -(-(x32.shape[1] + 1) // P) * P
    x8 = np.zeros((x32.shape[0], npad_h, x32.shape[2]), NP_F8)
    x8[:, :x32.shape[1], :] = x32.astype(NP_F8)
    w1 = np.asarray(inputs["W1"], np.float32)
    w2 = np.asarray(inputs["W2"], np.float32)
    w3 = np.asarray(inputs["W3"], np.float32)
    b1 = np.asarray(inputs["b1"], np.float32)
    b2 = np.asarray(inputs["b2"], np.float32)
    b3 = np.asarray(inputs["b3"], np.float32)
    e_sp = np.asarray(inputs["keypoint_line_without_temporal"]).astype(np.int64)
    e_tm = np.asarray(inputs["keypoint_line_with_temporal"]).astype(np.int64)

    b_total, n, c = x.shape
    bl = b_total // n_cores
    kt = -(-(n + 1) // P)
    npad = kt * P
    ct = c // P

    nc = _get_program(bl, n, c)

    npr = -(-n // 16) * 16
    at_sp = _tile_rows(
        _norm_adj_T(e_sp, n, npad, bias_row=True)[:, :npr].astype(NP_F8), kt)
    at_tm = _tile_rows(
        _norm_adj_T(e_tm, n, npad, bias_row=False)[:, :npr].astype(NP_F8), kt)
    shared = {
        "at_sp": at_sp,
        "at_tm": at_tm,
        "w1": _tile_rows(w1.astype(NP_F8), ct),
        "w2": _tile_rows(w2.astype(NP_F8), ct),
        "w3": _tile_rows(w3.astype(NP_F8), ct),
        "b1": np.ascontiguousarray(b1.reshape(ct, P).T),
        "b2": np.ascontiguousarray(b2.reshape(ct, P).T),
        "b3": np.ascontiguousarray(b3.astype(NP_F8)[None, :]),
        "ident": np.eye(P, dtype=NP_BF16),
    }
    in_maps = [
        {"x": np.ascontiguousarray(x[i * bl:(i + 1) * bl]),
         "x8": np.ascontiguousarray(x8[i * bl:(i + 1) * bl]), **shared}
        for i in range(n_cores)
    ]
    res = run_bass_kernel_spmd(nc, in_maps, core_ids=list(range(n_cores)),
                               trace=trace)
    out = np.concatenate(
        [np.asarray(r["out"]).astype(np.float32) for r in res.results], axis=0)
    return out, res


def kernel(**inputs) -> np.ndarray:
    out, _ = run(inputs, trace=False)
    return out
